# revision 33
# baseline (speedup 1.0000x reference)
"""AttnBlock (GroupNorm + single-head HWxHW attention + residual) on 8 trn2 cores.

Sharding: data-parallel over (batch, query-half): core i handles batch i//2,
query columns [ (i%2)*2048, (i%2+1)*2048 ).  The input for odd cores is
column-rotated on the host so every core's queries are columns 0:2048 of its
input (softmax over keys is permutation invariant, so k/v order doesn't
matter) -- this keeps the program SPMD (one NEFF for all 8 cores).

Device algorithm (per core, C=128 channels on partitions, N=4096 spatial):
  - GroupNorm stats: per-channel bn_stats/bn_aggr, then group (16-channel)
    reduction + broadcast via tiny mask matmuls on the PE.
  - h and the q/k projections are bf16; v is produced directly TRANSPOSED
    (vT[m,c]) and quantized to fp8e4 for the PV matmul.
  - Scores are computed transposed: sT[m-tile, n-block] = k_tile^T . q_blk
    (bf16 matmuls, 1 PE cycle/row).  exp() runs on the scalar engine straight
    out of PSUM over two banks at a time and writes fp8e4 directly; a fixed
    bias shift keeps exp values ~O(10) (max fp8e4 is 240), and cancels in
    num/den.
  - PV runs as fp8 DoubleRow matmuls (two 128-row contraction chunks per
    instruction): pv[c,n] += vT_pair^T . pt_pair over 16 pair-groups in PSUM.
    The softmax denominator rides along for free: wp is SVD-rotated on the
    host and its smallest singular direction dropped (error ~1e-4), freeing
    v-channel 127; vT's column 127 is all-ones, so pv row 127 accumulates
    sum_m(pt) = the denominator with no extra PE stream.
  - 1/den (fast Newton reciprocal on DVE) is broadcast across partitions by
    GPSIMD and fused into the PV-drain copy; the output projection then only
    needs a single residual add: out = x + wp.(num/den) [+ bias].

Host folding: gn_scale/gn_bias are folded into the q/k/v weights and biases;
the k bias is dropped entirely (additive per-query constant is softmax
invariant); the v bias is folded into the output projection bias since
softmax rows sum to 1.  Remaining biases are per-partition scalar adds, only
emitted when nonzero.
"""

import os
import sys
import types

if "/opt/trn_rl_repo" not in sys.path:
    sys.path.insert(0, "/opt/trn_rl_repo")

import numpy as np

B, C, H, W = 4, 128, 64, 64
N = H * W              # 4096 spatial positions
NQ = N // 2            # 2048 queries per core
NB = 512               # query block (columns per psum bank)
NBLK = NQ // NB        # 4 query blocks
MT = N // 128          # 32 key tiles
NCH = 4                # x/h chunking (1024 columns per chunk)
GROUPS = 8
GSIZE = C // GROUPS    # 16 channels per group
EPS = 1e-6
SCALE = float(C) ** -0.5
EXP_SHIFT = -1.5       # exp(s*SCALE + shift): cancels in num/den, keeps
                       # values well under fp8e4's max of 240
NG = MT // 2           # 16 pair-groups (2 key tiles each) per query block

# Which engine computes exp for each pair-group: 'A' = scalar (ACT, exact
# exp, fp8 out), 'D' = vector (DVE, Schraudolph-style linear-log exp written
# straight into the fp8e4 bit pattern via uint8).  Block 0 keeps more on ACT
# because the DVE is busy casting v^T tiles there.
EXP_SCHED_BLK0 = "AAAAAAADAAAAAAAD"
EXP_SCHED_BLKN = "AADAADAAADAADAAD"
# uint8 fast-exp: byte = round(A8*score + B8) approximates the e4m3 bit
# pattern of exp(SCALE*score + EXP_SHIFT).  56 = e4m3 exponent-bias offset
# (7*8), -0.44 = minimax mantissa correction (tuned numerically).
A8 = (8.0 / np.log(2.0)) * SCALE
B8 = 56.0 - 0.44 + (8.0 / np.log(2.0)) * EXP_SHIFT

LAST_RESULTS = None    # BassKernelResults of the most recent kernel() call


def _install_ntff_hook():
    """antenv.axon_hooks is missing from this container; inject it so
    run_bass_kernel_spmd(trace=True) can capture NTFF profiles."""
    if "antenv.axon_hooks" in sys.modules:
        return
    mod = types.ModuleType("antenv.axon_hooks")
    holder = [None]
    mod.set_axon_ntff_profile_hook = lambda h: holder.__setitem__(0, h)
    mod.get_axon_ntff_profile_hook = lambda: holder[0]
    sys.modules["antenv.axon_hooks"] = mod
    try:
        from trn_agent_boot.trn_boot import _ntff_profile_via_ctypes

        mod.set_axon_ntff_profile_hook(
            _ntff_profile_via_ctypes("/opt/axon/libaxon_pjrt.so")
        )
    except Exception:
        pass


_NC_CACHE = {}


def _build(use_bq: bool, use_bp: bool):
    key = (use_bq, use_bp)
    if key in _NC_CACHE:
        return _NC_CACHE[key]

    import concourse.bacc as bacc
    import concourse.mybir as mybir
    import concourse.tile as tile

    f32 = mybir.dt.float32
    bf16 = mybir.dt.bfloat16
    f8 = mybir.dt.float8e4
    u8 = mybir.dt.uint8
    DR = mybir.MatmulPerfMode.DoubleRow

    nc = bacc.Bacc("TRN2", target_bir_lowering=False, debug=False, num_devices=8)

    xp = nc.dram_tensor("xp", [C, N], f32, kind="ExternalInput")
    wqT_d = nc.dram_tensor("wqT", [C, C], bf16, kind="ExternalInput")
    wkT_d = nc.dram_tensor("wkT", [C, C], bf16, kind="ExternalInput")
    wvT_d = nc.dram_tensor("wvT", [C, C], bf16, kind="ExternalInput")
    wpT_d = nc.dram_tensor("wpT", [C, C], bf16, kind="ExternalInput")
    bq_d = nc.dram_tensor("bqe", [C, 1], f32, kind="ExternalInput")
    bp_d = nc.dram_tensor("bpe", [C, 1], f32, kind="ExternalInput")
    out_d = nc.dram_tensor("out", [C, NQ], f32, kind="ExternalOutput")

    # Group-mean reduction masks: gm averages a group's 16 channels (and the
    # spatial dim) into one row; gmT broadcasts group rows back to channels.
    gm_np = np.zeros((C, GROUPS), np.float32)
    gmT_np = np.zeros((GROUPS, C), np.float32)
    for ch in range(C):
        gm_np[ch, ch // GSIZE] = 1.0 / GSIZE
        gmT_np[ch // GSIZE, ch] = 1.0
    import ml_dtypes

    gm_d = nc.inline_tensor(gm_np.astype(ml_dtypes.bfloat16), "gmask")
    gmT_d = nc.inline_tensor(gmT_np.astype(ml_dtypes.bfloat16), "gmaskT")

    Exp = mybir.ActivationFunctionType.Exp
    Sqrt = mybir.ActivationFunctionType.Sqrt
    add_op = mybir.AluOpType.add
    sub_op = mybir.AluOpType.subtract
    mult_op = mybir.AluOpType.mult
    CHW = N // NCH  # 1024

    with tile.TileContext(nc) as tc:
        with (
            tc.tile_pool(name="big", bufs=1) as big,
            tc.tile_pool(name="wgt", bufs=1) as wgt,
            tc.tile_pool(name="ptile", bufs=8) as ptile,
            tc.tile_pool(name="small", bufs=2) as small,
            tc.tile_pool(name="ostage", bufs=3) as ostage,
            tc.tile_pool(name="ps_s", bufs=3, space="PSUM") as ps_s,
            tc.tile_pool(name="ps_pv", bufs=1, space="PSUM") as ps_pv,
            tc.tile_pool(name="ps_m", bufs=1, space="PSUM") as ps_m,
        ):
            # --- load inputs: x first on both HWDGE rings (stats gate
            # everything), then weights/masks behind them ---
            # x in 8 half-chunk DMAs alternating the two HWDGE rings so
            # bn_stats can start on the first 512 columns as soon as they
            # land (the gpsimd ring is SWDGE -- far too slow for bulk loads)
            xc = []
            for j in range(NCH):
                xj = big.tile([C, CHW], f32, tag=f"x{j}")
                for s in range(2):
                    eng = nc.sync if (2 * j + s) % 2 == 0 else nc.scalar
                    eng.dma_start(
                        out=xj[:, s * 512 : (s + 1) * 512],
                        in_=xp.ap()[:, j * CHW + s * 512 : j * CHW + (s + 1) * 512],
                    )
                xc.append(xj)
            gm_sb = wgt.tile([C, GROUPS], bf16, tag="gm")
            nc.sync.dma_start(out=gm_sb[:], in_=gm_d.ap())
            gmT_sb = wgt.tile([GROUPS, C], bf16, tag="gmT")
            nc.sync.dma_start(out=gmT_sb[:], in_=gmT_d.ap())
            w_q = wgt.tile([C, C], bf16, tag="wq")
            nc.sync.dma_start(out=w_q[:], in_=wqT_d.ap())
            w_k = wgt.tile([C, C], bf16, tag="wk")
            nc.scalar.dma_start(out=w_k[:], in_=wkT_d.ap())
            w_v = wgt.tile([C, C], bf16, tag="wv")
            nc.sync.dma_start(out=w_v[:], in_=wvT_d.ap())
            w_p = wgt.tile([C, C], bf16, tag="wp")
            nc.scalar.dma_start(out=w_p[:], in_=wpT_d.ap())
            if use_bq:
                bq_sb = wgt.tile([C, 1], f32, tag="bq")
                nc.sync.dma_start(out=bq_sb[:], in_=bq_d.ap())
            if use_bp:
                bp_sb = wgt.tile([C, 1], f32, tag="bp")
                nc.sync.dma_start(out=bp_sb[:], in_=bp_d.ap())
            eps_sb = wgt.tile([C, 1], f32, tag="eps")
            nc.vector.memset(eps_sb[:], EPS)
            esh_sb = wgt.tile([C, 1], f32, tag="esh")
            nc.vector.memset(esh_sb[:], EXP_SHIFT)
            # warm the Exp table set now (overlapped with the x DMAs); the
            # one Sqrt (sd below) loads into the second table slot during the
            # fill.  Nothing else may run on ACT or the sets thrash.
            warm = wgt.tile([1, 1], f32, tag="warm")
            nc.scalar.activation(out=warm[:], in_=eps_sb[0:1, :], func=Exp)

            # --- GroupNorm statistics ---
            stats = small.tile([C, 8, 6], f32, tag="stats")
            for j in range(8):
                nc.vector.bn_stats(
                    out=stats[:, j, :],
                    in_=xc[j // 2][:, (j % 2) * 512 : (j % 2) * 512 + 512],
                )
            mv = small.tile([C, 2], f32, tag="mv")
            nc.vector.bn_aggr(out=mv[:], in_=stats[:])
            # t2 = per-channel [mean, E[x^2]]; gm then averages over the group
            t2 = small.tile([C, 2], bf16, tag="t2")
            nc.vector.tensor_copy(out=t2[:, 0:1], in_=mv[:, 0:1])
            nc.vector.tensor_tensor(t2[:, 1:2], mv[:, 0:1], mv[:, 0:1], mult_op)
            nc.vector.tensor_tensor(t2[:, 1:2], t2[:, 1:2], mv[:, 1:2], add_op)
            psg = ps_m.tile([GROUPS, 2], f32, tag="m")
            nc.tensor.matmul(psg[:], lhsT=gm_sb[:], rhs=t2[:], start=True, stop=True)
            g2 = small.tile([GROUPS, 2], bf16, tag="g2")
            nc.vector.tensor_copy(out=g2[:], in_=psg[:])
            psb = ps_m.tile([C, 2], f32, tag="m")
            nc.tensor.matmul(psb[:], lhsT=gmT_sb[:], rhs=g2[:], start=True, stop=True)
            # mu = E[x]; var = E[x^2] - mu^2 ; rstd = 1/sqrt(var+eps)
            mu = small.tile([C, 1], f32, tag="mu")
            nc.vector.tensor_copy(out=mu[:], in_=psb[:, 0:1])
            var = small.tile([C, 1], f32, tag="var")
            nc.vector.tensor_tensor(var[:], mu[:], mu[:], mult_op)
            nc.vector.tensor_tensor(var[:], psb[:, 1:2], var[:], sub_op)
            sd = small.tile([C, 1], f32, tag="sd")
            nc.scalar.activation(out=sd[:], in_=var[:], func=Sqrt, bias=eps_sb[:])
            rstd = small.tile([C, 1], f32, tag="rstd")
            nc.vector.reciprocal_approx_fast(out=rstd[:], in_=sd[:])

            # h = (x - mu) * rstd in bf16, chunked, with each chunk's q/k
            # projections emitted immediately behind it so block-0 scores can
            # start as soon as chunk 0 clears the in-order DVE queue.
            hc = []
            qb = [None] * NBLK
            kc = [None] * 8
            for j in range(NCH):
                hj = big.tile([C, CHW], bf16, tag=f"h{j}")
                nc.vector.tensor_scalar(
                    hj[:], xc[j][:], mu[:], rstd[:], op0=sub_op, op1=mult_op
                )
                hc.append(hj)
                for s in range(2):
                    col = 2 * j + s
                    hs = hj[:, s * 512 : (s + 1) * 512]
                    if j < 2:
                        psq = ps_s.tile([C, 2, 512], f32, tag="s", name=f"psq{col}")
                        nc.tensor.matmul(
                            psq[:, 0, :], lhsT=w_q[:], rhs=hs, start=True, stop=True
                        )
                        qj = big.tile([C, NB], bf16, tag=f"q{col}")
                        if use_bq:
                            nc.vector.tensor_scalar_add(qj[:], psq[:, 0, :], bq_sb[:])
                        else:
                            nc.vector.tensor_copy(out=qj[:], in_=psq[:, 0, :])
                        qb[col] = qj
                    psk = ps_s.tile([C, 2, 512], f32, tag="s", name=f"psk{col}")
                    nc.tensor.matmul(
                        psk[:, 0, :], lhsT=w_k[:], rhs=hs, start=True, stop=True
                    )
                    kj = big.tile([C, 512], bf16, tag=f"k{col}")
                    nc.vector.tensor_copy(out=kj[:], in_=psk[:, 0, :])
                    kc[col] = kj

            def hpart(lo, width):
                j = lo // CHW
                assert lo + width <= (j + 1) * CHW
                return hc[j][:, lo - j * CHW : lo - j * CHW + width]

            def kpart(mi):
                return kc[mi // 4][:, (mi % 4) * 128 : (mi % 4) * 128 + 128]

            vT_sb = big.tile([128, MT, C], f8, tag="vt")
            # column 0 of every vT tile is all-ones: pv row 0 then
            # accumulates the softmax denominator during the PV matmuls.
            nc.vector.memset(vT_sb[:, :, 0:1], 1.0)

            def emit_vt_pair(g):
                # one pair of v^T tiles, emitted lazily inside attention
                # block 0.  Uses the ps_m slot only: strictly FIFO there (pso
                # comes later in program order), so no slot-hold deadlock is
                # possible.  Single drain copy for both tiles.
                psv = ps_m.tile([C, 2, 127], f32, tag="m", name=f"psv{g}")
                for u in range(2):
                    nc.tensor.matmul(
                        psv[:, u, :],
                        lhsT=hpart((2 * g + u) * 128, 128),
                        rhs=w_v[:, 0:127],
                        start=True,
                        stop=True,
                    )
                nc.vector.tensor_copy(
                    out=vT_sb[:, 2 * g : 2 * g + 2, 1:128], in_=psv[:]
                )

            # --- attention over query blocks ---
            # Each block's softmax-normalize + output-projection drain is
            # deferred into the NEXT block's group loop so the PE never
            # stalls on the rden->broadcast->hv chain: the DVE/GPSIMD part
            # is emitted at g==1, the PE part (pso) at g==4, by which point
            # the PE has several score matmuls queued ahead of it.
            drain_front_p = [None]
            drain_back_p = [None]

            def make_drain(jb, pv):
                st = {}

                def front():
                    # pv row 0 is the softmax denominator (ones column of vT)
                    rden = small.tile([1, NB], f32, tag="rden")
                    nc.vector.reciprocal_approx_fast(out=rden[:], in_=pv[0:1, :])
                    rb = ostage.tile([128, NB], f32, tag="rb")
                    nc.gpsimd.partition_broadcast(rb[:], rden[:])
                    # normalize during the PV drain (row 0 becomes 1.0; wp
                    # row 0 is zero on the host so it never leaks)
                    hv = ostage.tile([C, NB], bf16, tag="hv")
                    nc.vector.tensor_tensor(hv[:], pv[:], rb[:], mult_op)
                    st["hv"] = hv

                def back():
                    pso = ps_m.tile([C, NB], f32, tag="m")
                    nc.tensor.matmul(
                        pso[:], lhsT=w_p[:], rhs=st["hv"][:], start=True,
                        stop=True,
                    )
                    o1 = ostage.tile([C, NB], f32, tag="o1")
                    xblk = xc[jb // 2][:, (jb % 2) * 512 : (jb % 2) * 512 + 512]
                    nc.vector.tensor_tensor(o1[:], pso[:], xblk, add_op)
                    if use_bp:
                        nc.vector.tensor_scalar_add(o1[:], o1[:], bp_sb[:])
                    nc.sync.dma_start(
                        out=out_d[:, jb * NB : (jb + 1) * NB], in_=o1[:]
                    )

                return front, back

            for jb in range(NBLK):
                qs = qb[jb][:]
                pv = ps_pv.tile([C, NB], f32, tag="pv")
                pts = [None] * NG
                # software-pipelined by two groups: scores/exp for g are
                # emitted (and scheduled) ahead of group g-2's PV consumer so
                # neither the PE nor the exp engines ever starve.
                sched = EXP_SCHED_BLK0 if jb == 0 else EXP_SCHED_BLKN
                for g in range(NG + 2):
                    if g < NG:
                        ss = ps_s.tile([128, 2, NB], f32, tag="s")
                        for u in range(2):
                            nc.tensor.matmul(
                                ss[:, u, :],
                                lhsT=kpart(2 * g + u),
                                rhs=qs,
                                start=True,
                                stop=True,
                            )
                        pt = ptile.tile([128, 2, NB], f8, tag="pt")
                        if sched[g] == "A":
                            nc.scalar.activation(
                                out=pt[:], in_=ss[:], func=Exp, scale=SCALE,
                                bias=esh_sb[:],
                            )
                        else:
                            # DVE fast-exp: affine into the e4m3 bit pattern
                            nc.vector.tensor_scalar(
                                pt.bitcast(u8)[:], ss[:], float(A8), float(B8),
                                op0=mult_op, op1=add_op,
                            )
                        pts[g] = pt
                        if jb == 0:
                            emit_vt_pair(g)
                    if g == 1 and drain_front_p[0] is not None:
                        drain_front_p[0]()
                        drain_front_p[0] = None
                    if g == 4 and drain_back_p[0] is not None:
                        drain_back_p[0]()
                        drain_back_p[0] = None
                    if g < 2:
                        continue
                    c = g - 2
                    pt = pts[c]
                    pts[c] = None
                    nc.tensor.matmul(
                        pv[:],
                        lhsT=vT_sb[:, 2 * c : 2 * c + 2, :],
                        rhs=pt[:],
                        start=(c == 0),
                        stop=(c == NG - 1),
                        perf_mode=DR,
                    )
                drain_front_p[0], drain_back_p[0] = make_drain(jb, pv)
            # last block drains immediately
            drain_front_p[0]()
            drain_back_p[0]()

    nc.compile()
    _NC_CACHE[key] = nc
    return nc


def kernel(**inputs):
    global LAST_RESULTS
    _install_ntff_hook()
    import ml_dtypes
    from concourse.bass_utils import run_bass_kernel_spmd

    bf16 = ml_dtypes.bfloat16

    ins = {
        k: np.ascontiguousarray(np.asarray(v), dtype=np.float32)
        for k, v in inputs.items()
    }
    x = ins["x"]
    gs, gb = ins["gn_scale"], ins["gn_bias"]

    # Fold the GroupNorm affine into the q/k/v weights; pre-transpose all
    # weights into the [in_channel, out_channel] layout the PE wants.
    wq_e = ins["wq"] * gs[None, :]
    wk_e = ins["wk"] * gs[None, :]
    wv_e = ins["wv"] * gs[None, :]
    # Rotate v-space by the SVD of wp and drop the smallest singular
    # direction: frees v-channel 127 for the all-ones denominator row.
    U, S, Vt = np.linalg.svd(ins["wp"].astype(np.float64))
    wv2 = (Vt @ wv_e.astype(np.float64))[:127]          # 127 x C
    wp2 = U[:, :127] * S[:127]                          # C x 127
    # device layout: vT column 0 = ones (denominator row), v2 channels in
    # columns 1..127; wp row 0 = 0 so the den row doesn't leak
    wv2T = np.zeros((C, C), np.float32)
    wv2T[:, :127] = wv2.T
    wp2T = np.zeros((C, C), np.float32)
    wp2T[1:, :] = wp2.T
    wqT = np.ascontiguousarray(wq_e.T.astype(bf16))
    wkT = np.ascontiguousarray(wk_e.T.astype(bf16))
    wvT = np.ascontiguousarray(wv2T.astype(bf16))
    wpT = np.ascontiguousarray(wp2T.astype(bf16))
    bq_e = (ins["bq"] + ins["wq"] @ gb).reshape(C, 1)
    bv_e = ins["bv"] + ins["wv"] @ gb
    bp_e = (ins["bp"] + ins["wp"] @ bv_e).reshape(C, 1)
    use_bq = bool(np.any(bq_e))
    use_bp = bool(np.any(bp_e))

    nc = _build(use_bq, use_bp)

    in_maps = []
    for core in range(8):
        b, half = core // 2, core % 2
        xb = x[b].reshape(C, N)
        if half == 1:
            xb = np.concatenate([xb[:, NQ:], xb[:, :NQ]], axis=1)
        in_maps.append(
            {
                "xp": np.ascontiguousarray(xb),
                "wqT": wqT,
                "wkT": wkT,
                "wvT": wvT,
                "wpT": wpT,
                "bqe": bq_e,
                "bpe": bp_e,
            }
        )

    trace = os.environ.get("KERNEL_TRACE", "0") == "1"
    res = run_bass_kernel_spmd(nc, in_maps, core_ids=list(range(8)), trace=trace)
    LAST_RESULTS = res

    out = np.empty((B, C, N), np.float32)
    for core in range(8):
        b, half = core // 2, core % 2
        out[b, :, half * NQ : (half + 1) * NQ] = res.results[core]["out"]
    return out.reshape(B, C, N)[..., : N].reshape(B, C, H, W)


# revision 34
# speedup vs baseline: 1.0190x; 1.0190x over previous
"""AttnBlock (GroupNorm + single-head HWxHW attention + residual) on 8 trn2 cores.

Sharding: data-parallel over (batch, query-half): core i handles batch i//2,
query columns [ (i%2)*2048, (i%2+1)*2048 ).  The input for odd cores is
column-rotated on the host so every core's queries are columns 0:2048 of its
input (softmax over keys is permutation invariant, so k/v order doesn't
matter) -- this keeps the program SPMD (one NEFF for all 8 cores).

Device algorithm (per core, C=128 channels on partitions, N=4096 spatial):
  - GroupNorm stats: per-channel bn_stats/bn_aggr, then group (16-channel)
    reduction + broadcast via tiny mask matmuls on the PE.
  - h and the q/k projections are bf16; v is produced directly TRANSPOSED
    (vT[m,c]) and quantized to fp8e4 for the PV matmul.
  - Scores are computed transposed: sT[m-tile, n-block] = k_tile^T . q_blk
    (bf16 matmuls, 1 PE cycle/row).  exp() runs on the scalar engine straight
    out of PSUM over two banks at a time and writes fp8e4 directly; a fixed
    bias shift keeps exp values ~O(10) (max fp8e4 is 240), and cancels in
    num/den.
  - PV runs as fp8 DoubleRow matmuls (two 128-row contraction chunks per
    instruction): pv[c,n] += vT_pair^T . pt_pair over 16 pair-groups in PSUM.
    The softmax denominator rides along for free: wp is SVD-rotated on the
    host and its smallest singular direction dropped (error ~1e-4), freeing
    v-channel 127; vT's column 127 is all-ones, so pv row 127 accumulates
    sum_m(pt) = the denominator with no extra PE stream.
  - 1/den (fast Newton reciprocal on DVE) is broadcast across partitions by
    GPSIMD and fused into the PV-drain copy; the output projection then only
    needs a single residual add: out = x + wp.(num/den) [+ bias].

Host folding: gn_scale/gn_bias are folded into the q/k/v weights and biases;
the k bias is dropped entirely (additive per-query constant is softmax
invariant); the v bias is folded into the output projection bias since
softmax rows sum to 1.  Remaining biases are per-partition scalar adds, only
emitted when nonzero.
"""

import os
import sys
import types

if "/opt/trn_rl_repo" not in sys.path:
    sys.path.insert(0, "/opt/trn_rl_repo")

import numpy as np

B, C, H, W = 4, 128, 64, 64
N = H * W              # 4096 spatial positions
NQ = N // 2            # 2048 queries per core
NB = 512               # query block (columns per psum bank)
NBLK = NQ // NB        # 4 query blocks
MT = N // 128          # 32 key tiles
NCH = 4                # x/h chunking (1024 columns per chunk)
GROUPS = 8
GSIZE = C // GROUPS    # 16 channels per group
EPS = 1e-6
SCALE = float(C) ** -0.5
EXP_SHIFT = -1.5       # exp(s*SCALE + shift): cancels in num/den, keeps
                       # values well under fp8e4's max of 240
NG = MT // 2           # 16 pair-groups (2 key tiles each) per query block

# Which engine computes exp for each pair-group: 'A' = scalar (ACT, exact
# exp, fp8 out), 'D' = vector (DVE, Schraudolph-style linear-log exp written
# straight into the fp8e4 bit pattern via uint8).  Block 0 keeps more on ACT
# because the DVE is busy casting v^T tiles there.
EXP_SCHED_BLK0 = "AAAAAAADAAAAAAAD"
EXP_SCHED_BLKN = "AADAADAAADAADAAD"
# uint8 fast-exp: byte = round(A8*score + B8) approximates the e4m3 bit
# pattern of exp(SCALE*score + EXP_SHIFT).  56 = e4m3 exponent-bias offset
# (7*8), -0.44 = minimax mantissa correction (tuned numerically).
A8 = (8.0 / np.log(2.0)) * SCALE
B8 = 56.0 - 0.44 + (8.0 / np.log(2.0)) * EXP_SHIFT

LAST_RESULTS = None    # BassKernelResults of the most recent kernel() call


def _install_ntff_hook():
    """antenv.axon_hooks is missing from this container; inject it so
    run_bass_kernel_spmd(trace=True) can capture NTFF profiles."""
    if "antenv.axon_hooks" in sys.modules:
        return
    mod = types.ModuleType("antenv.axon_hooks")
    holder = [None]
    mod.set_axon_ntff_profile_hook = lambda h: holder.__setitem__(0, h)
    mod.get_axon_ntff_profile_hook = lambda: holder[0]
    sys.modules["antenv.axon_hooks"] = mod
    try:
        from trn_agent_boot.trn_boot import _ntff_profile_via_ctypes

        mod.set_axon_ntff_profile_hook(
            _ntff_profile_via_ctypes("/opt/axon/libaxon_pjrt.so")
        )
    except Exception:
        pass


_NC_CACHE = {}


def _build(use_bq: bool, use_bp: bool):
    key = (use_bq, use_bp)
    if key in _NC_CACHE:
        return _NC_CACHE[key]

    import concourse.bacc as bacc
    import concourse.mybir as mybir
    import concourse.tile as tile

    f32 = mybir.dt.float32
    bf16 = mybir.dt.bfloat16
    f8 = mybir.dt.float8e4
    u8 = mybir.dt.uint8
    DR = mybir.MatmulPerfMode.DoubleRow

    nc = bacc.Bacc("TRN2", target_bir_lowering=False, debug=False, num_devices=8)

    xp = nc.dram_tensor("xp", [C, N], f32, kind="ExternalInput")
    wqT_d = nc.dram_tensor("wqT", [C, C], bf16, kind="ExternalInput")
    wkT_d = nc.dram_tensor("wkT", [C, C], bf16, kind="ExternalInput")
    wvT_d = nc.dram_tensor("wvT", [C, C], bf16, kind="ExternalInput")
    wpT_d = nc.dram_tensor("wpT", [C, C], bf16, kind="ExternalInput")
    bq_d = nc.dram_tensor("bqe", [C, 1], f32, kind="ExternalInput")
    bp_d = nc.dram_tensor("bpe", [C, 1], f32, kind="ExternalInput")
    out_d = nc.dram_tensor("out", [C, NQ], f32, kind="ExternalOutput")

    # Group-mean reduction masks: gm averages a group's 16 channels (and the
    # spatial dim) into one row; gmT broadcasts group rows back to channels.
    gm_np = np.zeros((C, GROUPS), np.float32)
    gmT_np = np.zeros((GROUPS, C), np.float32)
    for ch in range(C):
        gm_np[ch, ch // GSIZE] = 1.0 / GSIZE
        gmT_np[ch // GSIZE, ch] = 1.0
    import ml_dtypes

    gm_d = nc.inline_tensor(gm_np.astype(ml_dtypes.bfloat16), "gmask")
    gmT_d = nc.inline_tensor(gmT_np.astype(ml_dtypes.bfloat16), "gmaskT")

    Exp = mybir.ActivationFunctionType.Exp
    Sqrt = mybir.ActivationFunctionType.Sqrt
    add_op = mybir.AluOpType.add
    sub_op = mybir.AluOpType.subtract
    mult_op = mybir.AluOpType.mult
    CHW = N // NCH  # 1024

    with tile.TileContext(nc) as tc:
        with (
            tc.tile_pool(name="big", bufs=1) as big,
            tc.tile_pool(name="wgt", bufs=1) as wgt,
            tc.tile_pool(name="ptile", bufs=8) as ptile,
            tc.tile_pool(name="small", bufs=2) as small,
            tc.tile_pool(name="ostage", bufs=3) as ostage,
            tc.tile_pool(name="ps_s", bufs=3, space="PSUM") as ps_s,
            tc.tile_pool(name="ps_pv", bufs=1, space="PSUM") as ps_pv,
            tc.tile_pool(name="ps_m", bufs=1, space="PSUM") as ps_m,
        ):
            # --- load inputs: x first on both HWDGE rings (stats gate
            # everything), then weights/masks behind them ---
            # x in 8 half-chunk DMAs alternating the two HWDGE rings so
            # bn_stats can start on the first 512 columns as soon as they
            # land (the gpsimd ring is SWDGE -- far too slow for bulk loads)
            xc = []
            for j in range(NCH):
                xj = big.tile([C, CHW], f32, tag=f"x{j}")
                for s in range(2):
                    eng = nc.sync if (2 * j + s) % 2 == 0 else nc.scalar
                    eng.dma_start(
                        out=xj[:, s * 512 : (s + 1) * 512],
                        in_=xp.ap()[:, j * CHW + s * 512 : j * CHW + (s + 1) * 512],
                    )
                xc.append(xj)
            gm_sb = wgt.tile([C, GROUPS], bf16, tag="gm")
            nc.sync.dma_start(out=gm_sb[:], in_=gm_d.ap())
            gmT_sb = wgt.tile([GROUPS, C], bf16, tag="gmT")
            nc.sync.dma_start(out=gmT_sb[:], in_=gmT_d.ap())
            w_q = wgt.tile([C, C], bf16, tag="wq")
            nc.sync.dma_start(out=w_q[:], in_=wqT_d.ap())
            w_k = wgt.tile([C, C], bf16, tag="wk")
            nc.scalar.dma_start(out=w_k[:], in_=wkT_d.ap())
            w_v = wgt.tile([C, C], bf16, tag="wv")
            nc.sync.dma_start(out=w_v[:], in_=wvT_d.ap())
            w_p = wgt.tile([C, C], bf16, tag="wp")
            nc.scalar.dma_start(out=w_p[:], in_=wpT_d.ap())
            if use_bq:
                bq_sb = wgt.tile([C, 1], f32, tag="bq")
                nc.sync.dma_start(out=bq_sb[:], in_=bq_d.ap())
            if use_bp:
                bp_sb = wgt.tile([C, 1], f32, tag="bp")
                nc.sync.dma_start(out=bp_sb[:], in_=bp_d.ap())
            eps_sb = wgt.tile([C, 1], f32, tag="eps")
            nc.vector.memset(eps_sb[:], EPS)
            esh_sb = wgt.tile([C, 1], f32, tag="esh")
            nc.vector.memset(esh_sb[:], EXP_SHIFT)
            # warm the Exp table set now (overlapped with the x DMAs); the
            # one Sqrt (sd below) loads into the second table slot during the
            # fill.  Nothing else may run on ACT or the sets thrash.
            warm = wgt.tile([1, 1], f32, tag="warm")
            nc.scalar.activation(out=warm[:], in_=eps_sb[0:1, :], func=Exp)

            # --- GroupNorm statistics ---
            stats = small.tile([C, 8, 6], f32, tag="stats")
            for j in range(8):
                nc.vector.bn_stats(
                    out=stats[:, j, :],
                    in_=xc[j // 2][:, (j % 2) * 512 : (j % 2) * 512 + 512],
                )
            mv = small.tile([C, 2], f32, tag="mv")
            nc.vector.bn_aggr(out=mv[:], in_=stats[:])
            # t2 = per-channel [mean, E[x^2]]; gm then averages over the group
            t2 = small.tile([C, 2], bf16, tag="t2")
            nc.vector.tensor_copy(out=t2[:, 0:1], in_=mv[:, 0:1])
            nc.vector.tensor_tensor(t2[:, 1:2], mv[:, 0:1], mv[:, 0:1], mult_op)
            nc.vector.tensor_tensor(t2[:, 1:2], t2[:, 1:2], mv[:, 1:2], add_op)
            psg = ps_m.tile([GROUPS, 2], f32, tag="m")
            nc.tensor.matmul(psg[:], lhsT=gm_sb[:], rhs=t2[:], start=True, stop=True)
            g2 = small.tile([GROUPS, 2], bf16, tag="g2")
            nc.vector.tensor_copy(out=g2[:], in_=psg[:])
            psb = ps_m.tile([C, 2], f32, tag="m")
            nc.tensor.matmul(psb[:], lhsT=gmT_sb[:], rhs=g2[:], start=True, stop=True)
            # mu = E[x]; var = E[x^2] - mu^2 ; rstd = 1/sqrt(var+eps)
            mu = small.tile([C, 1], f32, tag="mu")
            nc.vector.tensor_copy(out=mu[:], in_=psb[:, 0:1])
            var = small.tile([C, 1], f32, tag="var")
            nc.vector.tensor_tensor(var[:], mu[:], mu[:], mult_op)
            nc.vector.tensor_tensor(var[:], psb[:, 1:2], var[:], sub_op)
            sd = small.tile([C, 1], f32, tag="sd")
            nc.scalar.activation(out=sd[:], in_=var[:], func=Sqrt, bias=eps_sb[:])
            # the ACT has ONE active table set: the Sqrt above evicted Exp,
            # so re-warm Exp now (still fill time) or the first real exp
            # stalls 1.3us mid-attention on a table load
            nc.scalar.activation(out=warm[:], in_=eps_sb[0:1, :], func=Exp)
            rstd = small.tile([C, 1], f32, tag="rstd")
            nc.vector.reciprocal_approx_fast(out=rstd[:], in_=sd[:])

            # h = (x - mu) * rstd in bf16, chunked, with each chunk's q/k
            # projections emitted immediately behind it so block-0 scores can
            # start as soon as chunk 0 clears the in-order DVE queue.
            hc = []
            qb = [None] * NBLK
            kc = [None] * 8
            for j in range(NCH):
                hj = big.tile([C, CHW], bf16, tag=f"h{j}")
                nc.vector.tensor_scalar(
                    hj[:], xc[j][:], mu[:], rstd[:], op0=sub_op, op1=mult_op
                )
                hc.append(hj)
                for s in range(2):
                    col = 2 * j + s
                    hs = hj[:, s * 512 : (s + 1) * 512]
                    if j < 2:
                        psq = ps_s.tile([C, 2, 512], f32, tag="s", name=f"psq{col}")
                        nc.tensor.matmul(
                            psq[:, 0, :], lhsT=w_q[:], rhs=hs, start=True, stop=True
                        )
                        qj = big.tile([C, NB], bf16, tag=f"q{col}")
                        if use_bq:
                            nc.vector.tensor_scalar_add(qj[:], psq[:, 0, :], bq_sb[:])
                        else:
                            nc.vector.tensor_copy(out=qj[:], in_=psq[:, 0, :])
                        qb[col] = qj
                    psk = ps_s.tile([C, 2, 512], f32, tag="s", name=f"psk{col}")
                    nc.tensor.matmul(
                        psk[:, 0, :], lhsT=w_k[:], rhs=hs, start=True, stop=True
                    )
                    kj = big.tile([C, 512], bf16, tag=f"k{col}")
                    nc.vector.tensor_copy(out=kj[:], in_=psk[:, 0, :])
                    kc[col] = kj

            def hpart(lo, width):
                j = lo // CHW
                assert lo + width <= (j + 1) * CHW
                return hc[j][:, lo - j * CHW : lo - j * CHW + width]

            def kpart(mi):
                return kc[mi // 4][:, (mi % 4) * 128 : (mi % 4) * 128 + 128]

            vT_sb = big.tile([128, MT, C], f8, tag="vt")
            # column 0 of every vT tile is all-ones: pv row 0 then
            # accumulates the softmax denominator during the PV matmuls.
            nc.vector.memset(vT_sb[:, :, 0:1], 1.0)

            def emit_vt_pair(g):
                # one pair of v^T tiles, emitted lazily inside attention
                # block 0.  Uses the ps_m slot only: strictly FIFO there (pso
                # comes later in program order), so no slot-hold deadlock is
                # possible.  Single drain copy for both tiles.
                psv = ps_m.tile([C, 2, 127], f32, tag="m", name=f"psv{g}")
                for u in range(2):
                    nc.tensor.matmul(
                        psv[:, u, :],
                        lhsT=hpart((2 * g + u) * 128, 128),
                        rhs=w_v[:, 0:127],
                        start=True,
                        stop=True,
                    )
                nc.vector.tensor_copy(
                    out=vT_sb[:, 2 * g : 2 * g + 2, 1:128], in_=psv[:]
                )

            # --- attention over query blocks ---
            # Each block's softmax-normalize + output-projection drain is
            # deferred into the NEXT block's group loop so the PE never
            # stalls on the rden->broadcast->hv chain: the DVE/GPSIMD part
            # is emitted at g==1, the PE part (pso) at g==4, by which point
            # the PE has several score matmuls queued ahead of it.
            drain_front_p = [None]
            drain_back_p = [None]

            def make_drain(jb, pv):
                st = {}

                def front():
                    # pv row 0 is the softmax denominator (ones column of vT)
                    rden = small.tile([1, NB], f32, tag="rden")
                    nc.vector.reciprocal_approx_fast(out=rden[:], in_=pv[0:1, :])
                    rb = ostage.tile([128, NB], f32, tag="rb")
                    nc.gpsimd.partition_broadcast(rb[:], rden[:])
                    # normalize during the PV drain (row 0 becomes 1.0; wp
                    # row 0 is zero on the host so it never leaks)
                    hv = ostage.tile([C, NB], bf16, tag="hv")
                    nc.vector.tensor_tensor(hv[:], pv[:], rb[:], mult_op)
                    st["hv"] = hv

                def back():
                    pso = ps_m.tile([C, NB], f32, tag="m")
                    nc.tensor.matmul(
                        pso[:], lhsT=w_p[:], rhs=st["hv"][:], start=True,
                        stop=True,
                    )
                    o1 = ostage.tile([C, NB], f32, tag="o1")
                    xblk = xc[jb // 2][:, (jb % 2) * 512 : (jb % 2) * 512 + 512]
                    nc.vector.tensor_tensor(o1[:], pso[:], xblk, add_op)
                    if use_bp:
                        nc.vector.tensor_scalar_add(o1[:], o1[:], bp_sb[:])
                    nc.sync.dma_start(
                        out=out_d[:, jb * NB : (jb + 1) * NB], in_=o1[:]
                    )

                return front, back

            for jb in range(NBLK):
                qs = qb[jb][:]
                pv = ps_pv.tile([C, NB], f32, tag="pv")
                pts = [None] * NG
                # software-pipelined by two groups: scores/exp for g are
                # emitted (and scheduled) ahead of group g-2's PV consumer so
                # neither the PE nor the exp engines ever starve.
                sched = EXP_SCHED_BLK0 if jb == 0 else EXP_SCHED_BLKN
                for g in range(NG + 2):
                    if g < NG:
                        ss = ps_s.tile([128, 2, NB], f32, tag="s")
                        for u in range(2):
                            nc.tensor.matmul(
                                ss[:, u, :],
                                lhsT=kpart(2 * g + u),
                                rhs=qs,
                                start=True,
                                stop=True,
                            )
                        pt = ptile.tile([128, 2, NB], f8, tag="pt")
                        if sched[g] == "A":
                            nc.scalar.activation(
                                out=pt[:], in_=ss[:], func=Exp, scale=SCALE,
                                bias=esh_sb[:],
                            )
                        else:
                            # DVE fast-exp: affine into the e4m3 bit pattern
                            nc.vector.tensor_scalar(
                                pt.bitcast(u8)[:], ss[:], float(A8), float(B8),
                                op0=mult_op, op1=add_op,
                            )
                        pts[g] = pt
                        if jb == 0:
                            emit_vt_pair(g)
                    if g == 1 and drain_front_p[0] is not None:
                        drain_front_p[0]()
                        drain_front_p[0] = None
                    if g == 4 and drain_back_p[0] is not None:
                        drain_back_p[0]()
                        drain_back_p[0] = None
                    if g < 2:
                        continue
                    c = g - 2
                    pt = pts[c]
                    pts[c] = None
                    nc.tensor.matmul(
                        pv[:],
                        lhsT=vT_sb[:, 2 * c : 2 * c + 2, :],
                        rhs=pt[:],
                        start=(c == 0),
                        stop=(c == NG - 1),
                        perf_mode=DR,
                    )
                drain_front_p[0], drain_back_p[0] = make_drain(jb, pv)
            # last block drains immediately
            drain_front_p[0]()
            drain_back_p[0]()

    nc.compile()
    _NC_CACHE[key] = nc
    return nc


def kernel(**inputs):
    global LAST_RESULTS
    _install_ntff_hook()
    import ml_dtypes
    from concourse.bass_utils import run_bass_kernel_spmd

    bf16 = ml_dtypes.bfloat16

    ins = {
        k: np.ascontiguousarray(np.asarray(v), dtype=np.float32)
        for k, v in inputs.items()
    }
    x = ins["x"]
    gs, gb = ins["gn_scale"], ins["gn_bias"]

    # Fold the GroupNorm affine into the q/k/v weights; pre-transpose all
    # weights into the [in_channel, out_channel] layout the PE wants.
    wq_e = ins["wq"] * gs[None, :]
    wk_e = ins["wk"] * gs[None, :]
    wv_e = ins["wv"] * gs[None, :]
    # Rotate v-space by the SVD of wp and drop the smallest singular
    # direction: frees v-channel 127 for the all-ones denominator row.
    U, S, Vt = np.linalg.svd(ins["wp"].astype(np.float64))
    wv2 = (Vt @ wv_e.astype(np.float64))[:127]          # 127 x C
    wp2 = U[:, :127] * S[:127]                          # C x 127
    # device layout: vT column 0 = ones (denominator row), v2 channels in
    # columns 1..127; wp row 0 = 0 so the den row doesn't leak
    wv2T = np.zeros((C, C), np.float32)
    wv2T[:, :127] = wv2.T
    wp2T = np.zeros((C, C), np.float32)
    wp2T[1:, :] = wp2.T
    wqT = np.ascontiguousarray(wq_e.T.astype(bf16))
    wkT = np.ascontiguousarray(wk_e.T.astype(bf16))
    wvT = np.ascontiguousarray(wv2T.astype(bf16))
    wpT = np.ascontiguousarray(wp2T.astype(bf16))
    bq_e = (ins["bq"] + ins["wq"] @ gb).reshape(C, 1)
    bv_e = ins["bv"] + ins["wv"] @ gb
    bp_e = (ins["bp"] + ins["wp"] @ bv_e).reshape(C, 1)
    use_bq = bool(np.any(bq_e))
    use_bp = bool(np.any(bp_e))

    nc = _build(use_bq, use_bp)

    in_maps = []
    for core in range(8):
        b, half = core // 2, core % 2
        xb = x[b].reshape(C, N)
        if half == 1:
            xb = np.concatenate([xb[:, NQ:], xb[:, :NQ]], axis=1)
        in_maps.append(
            {
                "xp": np.ascontiguousarray(xb),
                "wqT": wqT,
                "wkT": wkT,
                "wvT": wvT,
                "wpT": wpT,
                "bqe": bq_e,
                "bpe": bp_e,
            }
        )

    trace = os.environ.get("KERNEL_TRACE", "0") == "1"
    res = run_bass_kernel_spmd(nc, in_maps, core_ids=list(range(8)), trace=trace)
    LAST_RESULTS = res

    out = np.empty((B, C, N), np.float32)
    for core in range(8):
        b, half = core // 2, core % 2
        out[b, :, half * NQ : (half + 1) * NQ] = res.results[core]["out"]
    return out.reshape(B, C, N)[..., : N].reshape(B, C, H, W)
